# revision 1
# baseline (speedup 1.0000x reference)
"""DimeNet++ interaction block on 8 Trainium2 NeuronCores (Bass/Tile).

Strategy:
- Sort triplets by idx_ji, shard by 25k-edge ranges per core (no all-reduce:
  each core owns its output edge slice).
- Within a core, order triplets by (window-block B, table-chunk c, window w)
  with fixed per-cell capacity so one SPMD program serves all cores.
- x_kj down-projection table [E,64] f32 computed E-sharded, AllGathered.
- Gather x_kj rows with dma_gather (int16 indices into 28672-row chunks).
- segment_sum via one-hot matmuls accumulated in PSUM windows of 256 edges.
"""
import sys
import os as _os2
import numpy as np

sys.path.insert(0, "/opt/trn_rl_repo")

N_CORES = 8
WIN_E = 256          # edges per PSUM window
CHUNK_ROWS = 1 << 30  # single chunk: indirect DMA takes int32 rows
TILE_T = 128
PHASES = "ABC"       # debug: subset of phases to build ("A", "AG", "AB", "ABC")
B_LEVEL = 3          # debug: 0=gather only, 1=+mm1, 2=+onehot/mult, 3=full


def _apply_tile_patches():
    """walrus in this container allows only 1 sync wait per instruction; split
    the TileContext tail drain into a chain of single-wait NOPs. Also register
    the NTFF profile hook so trace=True works (used by test harness only)."""
    import types
    import concourse.tile as tile
    from concourse.vector_clock import ScopedClock

    def _drain_and_barrier_split(self, tick_clock, wait_clock):
        gc = tick_clock.global_clock
        procs = [i for i in range(len(gc)) if gc[i] > 0]
        chunks = [procs[i : i + 1] for i in range(len(procs))]
        for ch in chunks[:-1] if chunks else []:
            nop = self.nc.sync.nop()
            pc = ScopedClock()
            for p in ch:
                pc.require_at_least(None, p, gc[p])
            wait_clock.add_sem_waits(nop.ins, pc)
        drain_inst = self.nc.sync.drain()
        pc = ScopedClock()
        for p in chunks[-1] if chunks else []:
            pc.require_at_least(None, p, gc[p])
        wait_clock.add_sem_waits(drain_inst.ins, pc)
        self.nc.all_engine_barrier()
        assert self.sems is not None
        popped = self.nc._tile_sem_poison_stack.pop()
        assert popped is self._sem_poison
        self.nc.clear_and_free_semaphores(list(self.sems.allocated().values()))
        self.nc.all_engine_barrier()

    tile.TileContext._drain_and_barrier = _drain_and_barrier_split

    if "antenv.axon_hooks" not in sys.modules:
        mod = types.ModuleType("antenv.axon_hooks")
        _state = {"hook": None}
        mod.set_axon_ntff_profile_hook = lambda h: _state.__setitem__("hook", h)
        mod.get_axon_ntff_profile_hook = lambda: _state["hook"]
        sys.modules["antenv.axon_hooks"] = mod
        import antenv

        antenv.axon_hooks = mod
        try:
            from trn_agent_boot.trn_boot import _ntff_profile_via_ctypes

            hook = _ntff_profile_via_ctypes("/opt/axon/libaxon_pjrt.so")
            if hook is not None:
                mod.set_axon_ntff_profile_hook(hook)
        except Exception:
            pass


def _build_program(E, H, I_DIM, NR, NS_NR, NB_BEFORE, NB_AFTER,
                   e_core, e_pad, wpc, wpb, n_chunk, cap_tiles):
    """Build the SPMD Bass program. Returns (nc, names)."""
    import concourse.bass as bass
    import concourse.bacc as bacc
    import concourse.tile as tile
    from concourse import mybir
    from contextlib import ExitStack

    f16, f32 = mybir.dt.float16, mybir.dt.float32
    i16 = mybir.dt.int16
    AF = mybir.ActivationFunctionType
    if _os2.environ.get("KRELU"):
        AF = type("AFX", (), {"Silu": mybir.ActivationFunctionType.Relu})
    OP = mybir.AluOpType

    n_blocks = wpc // wpb                 # window blocks per core
    n_calls = n_blocks * n_chunk          # gather calls per core
    cell_slots = cap_tiles * TILE_T       # token slots per (B,c,w) cell
    call_slots = wpb * cell_slots         # token slots per gather call
    n_cells = n_calls * wpb
    n_slots = n_cells * cell_slots
    e_tab = N_CORES * e_pad               # padded global table rows
    assert e_tab % n_chunk == 0 or e_tab <= n_chunk * CHUNK_ROWS
    NB512 = e_pad // 512                  # 512-edge blocks per core

    nc = bacc.Bacc("TRN2", target_bir_lowering=False, debug=False,
                   num_devices=N_CORES)

    def din(name, shape, dt):
        return nc.dram_tensor(name, shape, dt, kind="ExternalInput")

    # --- inputs (per core) ---
    xT_in = din("xT", [H, e_pad], f32)
    rbfT_in = din("rbfT", [NR, e_pad], f32)
    sbfT_in = din("sbfT", [n_cells, 64, cell_slots], f16)
    gidx_in = din("gidx", [n_calls, 128, wpb * cap_tiles], mybir.dt.int32)
    jil_in = din("jil", [n_calls, 128, wpb * cap_tiles], f32)
    iota_in = din("iota", [128, WIN_E], f16)
    ident_in = din("ident", [128, 128], f32)
    w_ji_in = din("w_ji", [H, H], f32)
    b_ji_in = din("b_ji", [H, 1], f32)
    w_kj_in = din("w_kj", [H, H], f32)
    b_kj_in = din("b_kj", [H, 1], f32)
    wc_rbf_in = din("wc_rbf", [NR, H], f32)
    wc_sbf_in = din("wc_sbf", [64, 64], f16)
    w_down_in = din("w_down", [H, I_DIM], f32)
    w_up_in = din("w_up", [64, H], f16)
    n_res = NB_BEFORE * 2 + 1 + NB_AFTER * 2
    w_res_in = din("w_res", [H, n_res * H], f32)  # rb0,rb1,...,lin,ra0... cols
    b_res_in = din("b_res", [H, n_res], f32)

    out_ext = nc.dram_tensor("out", [e_pad, H], f32, kind="ExternalOutput")

    with tile.TileContext(nc) as tc, ExitStack() as ctx:
        const = ctx.enter_context(tc.tile_pool(name="const", bufs=1))
        persist = ctx.enter_context(tc.tile_pool(name="persist", bufs=1))
        dram = ctx.enter_context(tc.tile_pool(name="dram", bufs=1, space="DRAM"))
        tab_slice = dram.tile([e_pad, 64], f32, tag="tab_slice")
        tab_full = dram.tile([e_tab, 64], f32, tag="tab_full")

        # constants / weights in SBUF
        def load_const(ap_in, shape, dt, tag):
            t = const.tile(shape, dt, tag=tag)
            nc.sync.dma_start(t[:], ap_in[:])
            return t

        iota = load_const(iota_in, [128, WIN_E], f16, "c_iota")
        ident = load_const(ident_in, [128, 128], f32, "c_ident")
        w_ji = load_const(w_ji_in, [H, H], f32, "c_wji")
        b_ji = load_const(b_ji_in, [H, 1], f32, "c_bji")
        w_kj = load_const(w_kj_in, [H, H], f32, "c_wkj")
        b_kj = load_const(b_kj_in, [H, 1], f32, "c_bkj")
        wc_rbf = load_const(wc_rbf_in, [NR, H], f32, "c_wcrbf")
        wc_sbf = load_const(wc_sbf_in, [64, 64], f16, "c_wcsbf")
        w_down = load_const(w_down_in, [H, I_DIM], f32, "c_wdown")
        w_up = load_const(w_up_in, [64, H], f16, "c_wup")
        w_res = load_const(w_res_in, [H, n_res * H], f32, "c_wres")
        b_res = load_const(b_res_in, [H, n_res], f32, "c_bres")

        x_jiT = persist.tile([H, e_pad], f16)      # silu(x@W_ji+b) transposed
        agg = persist.tile([64, e_pad], f16)       # segment sums (transposed)

        # ---------------- phase A: edge features + gather table ----------
        with (
            tc.tile_pool(name="a_sb", bufs=3) as a_sb,
            tc.tile_pool(name="a_ps", bufs=2, space="PSUM") as a_ps,
        ):
            for blk in range(0 if _os2.environ.get("SKIPA") else NB512):
                sl = slice(blk * 512, (blk + 1) * 512)
                xT = a_sb.tile([H, 512], f32, tag="xT")
                nc.sync.dma_start(xT[:], xT_in[:, sl])
                rbfT = a_sb.tile([NR, 512], f32, tag="rbfT")
                nc.sync.dma_start(rbfT[:], rbfT_in[:, sl])

                ps_ji = a_ps.tile([H, 512], f32, tag="psA")
                nc.tensor.matmul(out=ps_ji[:], lhsT=w_ji[:], rhs=xT[:],
                                 start=True, stop=True)
                nc.scalar.activation(x_jiT[:, sl], ps_ji[:], AF.Silu,
                                     bias=b_ji[:])

                ps_kj = a_ps.tile([H, 512], f32, tag="psA")
                nc.tensor.matmul(out=ps_kj[:], lhsT=w_kj[:], rhs=xT[:],
                                 start=True, stop=True)
                t1 = a_sb.tile([H, 512], f32, tag="t1")
                nc.scalar.activation(t1[:], ps_kj[:], AF.Silu, bias=b_kj[:])

                ps_rbf = a_ps.tile([H, 512], f32, tag="psA")
                nc.tensor.matmul(out=ps_rbf[:], lhsT=wc_rbf[:], rhs=rbfT[:],
                                 start=True, stop=True)
                t2 = a_sb.tile([H, 512], f32, tag="t2")
                nc.vector.tensor_tensor(out=t2[:], in0=t1[:], in1=ps_rbf[:],
                                        op=OP.mult)
                for j in range(4):
                    ps_d = a_ps.tile([128, 64], f32, tag="psD")
                    nc.tensor.matmul(out=ps_d[:],
                                     lhsT=t2[:, j * 128:(j + 1) * 128],
                                     rhs=w_down[:], start=True, stop=True)
                    td = a_sb.tile([128, 64], f32, tag="td")
                    nc.scalar.activation(td[:], ps_d[:], AF.Silu)
                    r0 = blk * 512 + j * 128
                    nc.sync.dma_start(tab_slice[r0:r0 + 128, :], td[:])

        # ---------------- AllGather the table ----------------------------
        import os as _os
        if _os.environ.get("NOCC"):
            with tc.tile_pool(name="ncc", bufs=2) as ncc:
                for blk in range(NB512):
                    d = ncc.tile([128, 4, 64], f32, tag="ncc")
                    nc.sync.dma_start(
                        d[:], tab_slice[blk * 512:(blk + 1) * 512, :].rearrange(
                            "(b p) d -> p b d", p=128))
                    nc.sync.dma_start(
                        tab_full[blk * 512:(blk + 1) * 512, :].rearrange(
                            "(b p) d -> p b d", p=128), d[:])
        elif "G" in PHASES or "B" in PHASES:
            nc.gpsimd.collective_compute(
                "AllGather", OP.bypass,
                replica_groups=[list(range(N_CORES))],
                ins=[tab_slice.opt()],
                outs=[tab_full.opt()],
            )

        if PHASES in ("A", "AG"):
            with tc.tile_pool(name="dbg", bufs=2) as dbg:
                src = tab_full if PHASES == "AG" else tab_slice
                for blk in range(NB512):
                    d = dbg.tile([128, 4, 64], f32, tag="dbg")
                    nc.sync.dma_start(
                        d[:], src[blk * 512:(blk + 1) * 512, :].rearrange(
                            "(b p) d -> p b d", p=128))
                    nc.sync.dma_start(
                        out_ext[blk * 512:(blk + 1) * 512, :64].rearrange(
                            "(b p) d -> p b d", p=128), d[:])

        # ---------------- phase B: triplets ------------------------------
        if "B" in PHASES:
            _phase_b(nc, tc, tile, mybir, n_blocks, n_chunk, wpb, cap_tiles,
                     call_slots, cell_slots, e_tab, sbfT_in, gidx_in, jil_in,
                     tab_full, wc_sbf, iota, agg)
        if "C" in PHASES:
            _phase_c(nc, tc, tile, mybir, NB512, H, n_res, NB_BEFORE, NB_AFTER,
                     w_up, w_res, b_res, agg, x_jiT, xT_in, ident, out_ext)

    nc.compile()
    return nc


def _phase_b(nc, tc, tile, mybir, n_blocks, n_chunk, wpb, cap_tiles,
             call_slots, cell_slots, e_tab, sbfT_in, gidx_in, jil_in,
             tab_full, wc_sbf, iota, agg):
    import concourse.bass as bass
    f16, f32 = mybir.dt.float16, mybir.dt.float32
    i16 = mybir.dt.int16
    OP = mybir.AluOpType
    with (
        tc.tile_pool(name="b_sb", bufs=3) as b_sb,
        tc.tile_pool(name="b_gat", bufs=6) as b_gat,
        tc.tile_pool(name="b_meta", bufs=2) as b_meta,
        tc.tile_pool(name="b_mega", bufs=2, space="PSUM") as b_mega,
        tc.tile_pool(name="b_ps", bufs=2, space="PSUM") as b_ps,
    ):
        for B in range(n_blocks):
            mega = b_sb.tile([64, wpb * WIN_E], f32, tag="megs")
            gats = []
            jils = []
            for c in range(n_chunk):
                call = B * n_chunk + c
                gi = b_meta.tile([128, wpb * cap_tiles], mybir.dt.int32,
                                 tag="gi")
                nc.sync.dma_start(gi[:], gidx_in[call])
                gats.append(gi)
                jl = b_meta.tile([128, wpb * cap_tiles], f32, tag="jl")
                nc.sync.dma_start(jl[:], jil_in[call])
                jils.append(jl)
            for c in range(n_chunk):
                for w in range(wpb):
                    cell = (B * n_chunk + c) * wpb + w
                    if B_LEVEL >= 1:
                        sbfT = b_sb.tile([64, cell_slots], f16, tag="sbfT")
                        nc.sync.dma_start(sbfT[:], sbfT_in[cell])
                    for t in range(cap_tiles):
                        col = w * cap_tiles + t
                        if B_LEVEL < 1:
                            continue
                        ps_se = b_ps.tile([128, 64], f32, tag="se")
                        nc.tensor.matmul(
                            out=ps_se[:],
                            lhsT=sbfT[:, t * 128:(t + 1) * 128],
                            rhs=wc_sbf[:], start=True, stop=True)
                        if B_LEVEL < 2:
                            continue
                        oh = b_sb.tile([128, WIN_E], f16, tag="oh")
                        nc.vector.tensor_scalar(
                            out=oh[:], in0=iota[:],
                            scalar1=jils[c][:, col:col + 1],
                            scalar2=None, op0=OP.is_equal)
                        gat = b_gat.tile([128, 64], f32, tag="gat")
                        nc.gpsimd.indirect_dma_start(
                            out=gat[:], out_offset=None,
                            in_=tab_full[:, :],
                            in_offset=bass.IndirectOffsetOnAxis(
                                ap=gats[c][:, col:col + 1], axis=0))
                        m = b_sb.tile([128, 64], f16, tag="m")
                        nc.vector.tensor_tensor(
                            out=m[:], in0=gat[:],
                            in1=ps_se[:], op=OP.mult)
                        if B_LEVEL < 3:
                            continue
                        ps_sc = b_mega.tile([64, WIN_E], f32, tag="sc")
                        nc.tensor.matmul(
                            out=ps_sc[:], lhsT=m[:], rhs=oh[:],
                            start=True, stop=True)
                        dst = mega[:, w * WIN_E:(w + 1) * WIN_E]
                        if c == 0 and t == 0:
                            nc.vector.tensor_copy(dst, ps_sc[:])
                        else:
                            nc.vector.tensor_tensor(
                                out=dst, in0=dst, in1=ps_sc[:], op=OP.add)
            e0 = B * wpb * WIN_E
            if B_LEVEL >= 3:
                nc.vector.tensor_copy(agg[:, e0:e0 + wpb * WIN_E], mega[:])


def _phase_c(nc, tc, tile, mybir, NB512, H, n_res, NB_BEFORE, NB_AFTER,
             w_up, w_res, b_res, agg, x_jiT, xT_in, ident, out_ext):
    f32 = mybir.dt.float32
    AF = mybir.ActivationFunctionType
    if _os2.environ.get("KRELU"):
        AF = type("AFX", (), {"Silu": mybir.ActivationFunctionType.Relu})
    OP = mybir.AluOpType
    with (
        tc.tile_pool(name="c_sb", bufs=3) as c_sb,
        tc.tile_pool(name="c_ps", bufs=2, space="PSUM") as c_ps,
    ):
        for blk in range(NB512):
            sl = slice(blk * 512, (blk + 1) * 512)
            ps_u = c_ps.tile([H, 512], f32, tag="psC")
            nc.tensor.matmul(out=ps_u[:], lhsT=w_up[:], rhs=agg[:, sl],
                             start=True, stop=True)
            su = c_sb.tile([H, 512], f32, tag="su")
            nc.scalar.activation(su[:], ps_u[:], AF.Silu)
            h = c_sb.tile([H, 512], f32, tag="h")
            nc.vector.tensor_tensor(out=h[:], in0=x_jiT[:, sl], in1=su[:],
                                    op=OP.add)

            def res_layer(h_in, li):
                ps_a = c_ps.tile([H, 512], f32, tag="psC")
                nc.tensor.matmul(out=ps_a[:],
                                 lhsT=w_res[:, li * H:(li + 1) * H],
                                 rhs=h_in[:], start=True, stop=True)
                inner = c_sb.tile([H, 512], f32, tag="inner")
                nc.scalar.activation(inner[:], ps_a[:], AF.Silu,
                                     bias=b_res[:, li:li + 1])
                ps_b = c_ps.tile([H, 512], f32, tag="psC")
                nc.tensor.matmul(out=ps_b[:],
                                 lhsT=w_res[:, (li + 1) * H:(li + 2) * H],
                                 rhs=inner[:], start=True, stop=True)
                s = c_sb.tile([H, 512], f32, tag="s")
                nc.scalar.activation(s[:], ps_b[:], AF.Silu,
                                     bias=b_res[:, li + 1:li + 2])
                h_out = c_sb.tile([H, 512], f32, tag="h")
                nc.vector.tensor_tensor(out=h_out[:], in0=h_in[:],
                                        in1=s[:], op=OP.add)
                return h_out

            li = 0
            for _ in range(NB_BEFORE):
                h = res_layer(h, li)
                li += 2
            ps_l = c_ps.tile([H, 512], f32, tag="psC")
            nc.tensor.matmul(out=ps_l[:],
                             lhsT=w_res[:, li * H:(li + 1) * H],
                             rhs=h[:], start=True, stop=True)
            sl_t = c_sb.tile([H, 512], f32, tag="s")
            nc.scalar.activation(sl_t[:], ps_l[:], AF.Silu,
                                 bias=b_res[:, li:li + 1])
            li += 1
            xT = c_sb.tile([H, 512], f32, tag="xT2")
            nc.sync.dma_start(xT[:], xT_in[:, sl])
            h = c_sb.tile([H, 512], f32, tag="h")
            nc.vector.tensor_tensor(out=h[:], in0=sl_t[:], in1=xT[:],
                                    op=OP.add)
            for _ in range(NB_AFTER):
                h = res_layer(h, li)
                li += 2

            for j in range(4):
                ps_t = c_ps.tile([128, 128], f32, tag="psT")
                nc.tensor.matmul(out=ps_t[:],
                                 lhsT=h[:, j * 128:(j + 1) * 128],
                                 rhs=ident[:], start=True, stop=True)
                ot = c_sb.tile([128, 128], f32, tag="ot")
                nc.vector.tensor_copy(ot[:], ps_t[:])
                r0 = blk * 512 + j * 128
                nc.sync.dma_start(out_ext[r0:r0 + 128, :], ot[:])


def kernel(**inputs):
    _apply_tile_patches()
    from concourse.bass_utils import run_bass_kernel_spmd

    x = np.asarray(inputs["x"], np.float32)
    rbf = np.asarray(inputs["rbf"], np.float32)
    sbf = np.asarray(inputs["sbf"], np.float32)
    idx_kj = np.asarray(inputs["idx_kj"]).astype(np.int64)
    idx_ji = np.asarray(inputs["idx_ji"]).astype(np.int64)

    E, H = x.shape
    T, NS_NR = sbf.shape
    NR = rbf.shape[1]
    I_DIM = np.asarray(inputs["W_down"]).shape[1]
    W_res_before = np.asarray(inputs["W_res_before"], np.float32)
    W_res_after = np.asarray(inputs["W_res_after"], np.float32)
    b_res_before = np.asarray(inputs["b_res_before"], np.float32)
    b_res_after = np.asarray(inputs["b_res_after"], np.float32)
    NB_BEFORE = W_res_before.shape[0]
    NB_AFTER = W_res_after.shape[0]

    assert E % N_CORES == 0
    e_core = E // N_CORES
    e_pad = -(-e_core // 512) * 512
    wpc = e_pad // WIN_E
    wpb = max(d for d in range(1, 8) if wpc % d == 0)
    e_tab = N_CORES * e_pad
    n_chunk = -(-e_tab // CHUNK_ROWS)

    # ---------------- host-side index preprocessing -------------------
    order = np.argsort(idx_ji, kind="stable")
    ji_sorted = idx_ji[order]
    bounds = np.searchsorted(ji_sorted, np.arange(N_CORES + 1) * e_core)

    # global padded table row for each triplet's kj edge
    kj_pad_all = (idx_kj // e_core) * e_pad + idx_kj % e_core

    n_blocks = wpc // wpb

    # first pass: find max cell count across cores to set cap_tiles
    percore = []
    max_cell = 0
    for k in range(N_CORES):
        oj = order[bounds[k]:bounds[k + 1]]
        ji_l = ji_sorted[bounds[k]:bounds[k + 1]] - k * e_core
        kj_g = kj_pad_all[oj]
        w = ji_l // WIN_E
        B = w // wpb
        c = kj_g // CHUNK_ROWS
        cell = (B * n_chunk + c) * wpb + (w % wpb)
        ord2 = np.argsort(cell, kind="stable")
        cell_s = cell[ord2]
        n_cells = n_blocks * n_chunk * wpb
        counts = np.bincount(cell_s, minlength=n_cells)
        max_cell = max(max_cell, int(counts.max()) if len(counts) else 0)
        percore.append((oj[ord2], ji_l[ord2], kj_g[ord2], cell_s, counts))

    cap_tiles = max(1, -(-max_cell // TILE_T))
    cell_slots = cap_tiles * TILE_T
    n_cells = n_blocks * n_chunk * wpb
    n_calls = n_blocks * n_chunk
    call_slots = wpb * cell_slots
    n_slots = n_cells * cell_slots

    sbf16 = np.zeros((T, 64), np.float16)
    sbf16[:, :NS_NR] = sbf.astype(np.float16)

    in_maps = []
    for k in range(N_CORES):
        oj, ji_l, kj_g, cell_s, counts = percore[k]
        starts = np.zeros(n_cells, np.int64)
        np.cumsum(counts[:-1], out=starts[1:])
        rank = np.arange(len(cell_s)) - starts[cell_s]
        slots = cell_s * cell_slots + rank

        sbf_slots = np.zeros((n_slots, 64), np.float16)
        sbf_slots[slots] = sbf16[oj]
        sbfT = np.ascontiguousarray(
            sbf_slots.reshape(n_cells, cell_slots, 64).transpose(0, 2, 1))

        gidx_flat = np.zeros(n_slots, np.int32)
        gidx_flat[slots] = kj_g.astype(np.int32)
        g = gidx_flat.reshape(n_calls, wpb * cap_tiles, TILE_T)
        gidx = np.ascontiguousarray(g.transpose(0, 2, 1))

        jil_flat = np.full(n_slots, -4096.0, np.float32)
        jil_flat[slots] = (ji_l % WIN_E).astype(np.float32)
        jl = jil_flat.reshape(n_calls, wpb * cap_tiles, TILE_T)
        jil = np.ascontiguousarray(jl.transpose(0, 2, 1))

        xT = np.zeros((H, e_pad), np.float32)
        xT[:, :e_core] = x[k * e_core:(k + 1) * e_core].T
        rbfT = np.zeros((NR, e_pad), np.float32)
        rbfT[:, :e_core] = rbf[k * e_core:(k + 1) * e_core].T

        wc_sbf = np.zeros((64, 64), np.float16)
        wc_sbf[:NS_NR] = (np.asarray(inputs["W_sbf1"], np.float32)
                          @ np.asarray(inputs["W_sbf2"], np.float32)
                          ).astype(np.float16)
        w_res = np.concatenate([
            W_res_before.reshape(-1, H, H),
            np.asarray(inputs["W_lin"], np.float32)[None],
            W_res_after.reshape(-1, H, H)])
        w_res = np.ascontiguousarray(
            w_res.transpose(1, 0, 2).reshape(H, -1))        # [H, n_res*H]
        b_res = np.concatenate([
            b_res_before.reshape(-1, H),
            np.asarray(inputs["b_lin"], np.float32)[None],
            b_res_after.reshape(-1, H)])
        b_res = np.ascontiguousarray(b_res.T)               # [H, n_res]

        in_maps.append({
            "xT": xT, "rbfT": rbfT, "sbfT": sbfT, "gidx": gidx, "jil": jil,
            "iota": np.tile(np.arange(WIN_E, dtype=np.float16)[None], (128, 1)),
            "ident": np.eye(128, dtype=np.float32),
            "w_ji": np.asarray(inputs["W_ji"], np.float32),
            "b_ji": np.asarray(inputs["b_ji"], np.float32)[:, None],
            "w_kj": np.asarray(inputs["W_kj"], np.float32),
            "b_kj": np.asarray(inputs["b_kj"], np.float32)[:, None],
            "wc_rbf": (np.asarray(inputs["W_rbf1"], np.float32)
                       @ np.asarray(inputs["W_rbf2"], np.float32)),
            "wc_sbf": wc_sbf,
            "w_down": np.asarray(inputs["W_down"], np.float32),
            "w_up": np.asarray(inputs["W_up"], np.float32).astype(np.float16),
            "w_res": w_res,
            "b_res": b_res,
        })

    nc = _build_program(E, H, I_DIM, NR, NS_NR, NB_BEFORE, NB_AFTER,
                        e_core, e_pad, wpc, wpb, n_chunk, cap_tiles)
    import os as _os
    if _os.environ.get("KSIM"):
        from concourse import bass_interp
        sim = bass_interp.MultiCoreSim(nc, N_CORES)
        for i in range(N_CORES):
            for name, arr in in_maps[i].items():
                sim.cores[i].tensor(name)[:] = arr
        sim.simulate()
        out = np.empty((E, H), np.float32)
        for k in range(N_CORES):
            out[k * e_core:(k + 1) * e_core] = \
                np.asarray(sim.cores[k].tensor("out"))[:e_core]
        return out
    if _os.environ.get("KDUMP"):
        for bb in nc.m.functions[0].blocks:
            for inst in bb.instructions:
                nm = type(inst).__name__
                if "GatherAnt" in nm or "TriggerDma" in nm:
                    si = inst.sync_info
                    print(nm, inst.name, "eng=", inst.engine)
                    if si:
                        for u in si.on_update:
                            print("   upd:", u)
                        for w in si.on_wait:
                            print("   wait:", w)
        raise SystemExit(0)
    res = run_bass_kernel_spmd(nc, in_maps, list(range(N_CORES)),
                               trace=bool(_os2.environ.get("KTRACE")))
    if res.exec_time_ns is not None:
        print(f"HW exec time: {res.exec_time_ns} ns")

    out = np.empty((E, H), np.float32)
    for k in range(N_CORES):
        out[k * e_core:(k + 1) * e_core] = res.results[k]["out"][:e_core]
    return out

